# revision 1
# baseline (speedup 1.0000x reference)
"""Trainium2 Bass kernel for a 2-layer LSTM extractor.

Reference computation (see problem):
  x: [512, 1, 512, 28] -> squeeze -> [B=512, T=512, D=28]
  layer0: LSTM(D=28 -> H=128), layer1: LSTM(128 -> 128)
  output: final hidden state of layer1, [512, 128]

Strategy:
  - Data parallel: batch 512 sharded 8 ways -> B=64 per NeuronCore.
  - Per core, both layers fused in one time loop, layer1 skewed one step
    behind layer0 so its work fills engine gaps.
  - Gate-transposed layout everywhere: states h/c stored [H=128 part, B=64
    free]; gate pre-activations computed as [4H-chunk part, B free] via
    matmuls with stationary weight chunks lhsT=[K,128] and moving rhs=h.
    No per-step transposes anywhere.
  - L0 biases folded into the x-projection via a ones-row augmentation
    (K=29).  L1 biases applied via the ACT per-partition bias operand.
  - x is transposed on-chip (PE transpose) into [29, B*T] quarters;
    the per-step x-projection rhs is a strided column view.
"""

import os
import sys

import numpy as np

for _p in ("/opt/trn_rl_repo", os.path.expanduser("~/.axon_site/_ro/trn_rl_repo")):
    if os.path.isdir(_p) and _p not in sys.path:
        sys.path.insert(0, _p)

import concourse.bacc as bacc
import concourse.tile as tile
from concourse import masks, mybir
from concourse.bass_utils import run_bass_kernel_spmd

B_FULL, T_FULL, D, H = 512, 512, 28, 128
NCORES = 8
B = B_FULL // NCORES  # 64 per core
G4 = 4 * H  # 512
P = 128
F32 = mybir.dt.float32
AF = mybir.ActivationFunctionType

# weight chunk g (PyTorch gate order i,f,g,o) -> psum column block.
# Sigmoid gates (i,f,o) are kept contiguous so one ACT op covers them.
COL_OF = [0, 1, 3, 2]  # i->0, f->1, g->3, o->2
KA = 33  # augmented contraction dim for the L0 x-projection (28 x + pad + bias)


def _emit(nc, tc, t_steps):
    ctx = tc.octx if hasattr(tc, "octx") else None
    Q = 4 if t_steps % 4 == 0 and t_steps >= 4 else 1
    TQ = t_steps // Q

    x = nc.dram_tensor("x", [B, t_steps, D], F32, kind="ExternalInput").ap()
    wih0 = nc.dram_tensor("W_ih0", [G4, D], F32, kind="ExternalInput").ap()
    whh0 = nc.dram_tensor("W_hh0", [G4, H], F32, kind="ExternalInput").ap()
    bih0 = nc.dram_tensor("b_ih0", [1, G4], F32, kind="ExternalInput").ap()
    bhh0 = nc.dram_tensor("b_hh0", [1, G4], F32, kind="ExternalInput").ap()
    wih1 = nc.dram_tensor("W_ih1", [G4, H], F32, kind="ExternalInput").ap()
    whh1 = nc.dram_tensor("W_hh1", [G4, H], F32, kind="ExternalInput").ap()
    bih1 = nc.dram_tensor("b_ih1", [4, H], F32, kind="ExternalInput").ap()
    bhh1 = nc.dram_tensor("b_hh1", [4, H], F32, kind="ExternalInput").ap()
    out = nc.dram_tensor("out", [B, H], F32, kind="ExternalOutput").ap()

    from contextlib import ExitStack

    es = ExitStack()
    with es:
        consts = es.enter_context(tc.tile_pool(name="consts", bufs=1))
        wstage = es.enter_context(tc.tile_pool(name="wstage", bufs=2))
        xload = es.enter_context(tc.tile_pool(name="xload", bufs=4))
        pswt = es.enter_context(tc.tile_pool(name="pswt", bufs=2, space="PSUM"))
        psx = es.enter_context(tc.tile_pool(name="psx", bufs=2, space="PSUM"))
        ps0p = es.enter_context(tc.tile_pool(name="ps0p", bufs=2, space="PSUM"))
        ps1p = es.enter_context(tc.tile_pool(name="ps1p", bufs=2, space="PSUM"))
        states = es.enter_context(tc.tile_pool(name="states", bufs=3))
        work = es.enter_context(tc.tile_pool(name="work", bufs=3))

        ident = consts.tile([P, P], F32)
        masks.make_identity(nc, ident[:])

        # ---- weight prep: transposed lhsT chunks ----
        whh0T = consts.tile([P, G4], F32)
        wih1T = consts.tile([P, G4], F32)
        whh1T = consts.tile([P, G4], F32)
        for src, dst in ((whh0, whh0T), (wih1, wih1T), (whh1, whh1T)):
            for g in range(4):
                wst = wstage.tile([P, H], F32, tag="wst")
                nc.sync.dma_start(out=wst[:], in_=src[g * P : (g + 1) * P, :])
                pst = pswt.tile([P, P], F32, tag="pswt")
                nc.tensor.transpose(pst[:], wst[:], ident[:])
                nc.scalar.copy(out=dst[:, g * P : (g + 1) * P], in_=pst[:])

        # wih0T augmented with the summed L0 bias as row 32 (compute-op start
        # partitions must be 32-aligned); rows 28..31 are zero so the
        # matching garbage rows of xT contribute nothing. K = KA = 33.
        wih0T = consts.tile([KA, G4], F32)
        nc.vector.memset(wih0T[:], 0.0)
        for g in range(4):
            wst = wstage.tile([P, D], F32, tag="wst")
            nc.sync.dma_start(out=wst[:], in_=wih0[g * P : (g + 1) * P, :])
            pst = psx.tile([D, P], F32, tag="psx")
            nc.tensor.transpose(pst[:], wst[:], ident[:])
            nc.vector.tensor_copy(wih0T[0:D, g * P : (g + 1) * P], pst[:])
        b0a = work.tile([1, G4], F32, tag="b0a")
        b0b = work.tile([1, G4], F32, tag="b0b")
        b0sum = work.tile([1, G4], F32, tag="b0sum")
        nc.sync.dma_start(out=b0a[:], in_=bih0)
        nc.sync.dma_start(out=b0b[:], in_=bhh0)
        nc.vector.tensor_add(b0sum[:], b0a[:], b0b[:])
        nc.sync.dma_start(out=wih0T[KA - 1 : KA, :], in_=b0sum[:])

        # L1 bias as per-partition columns: b1T[p, g] = bias of gate-chunk g
        b1a = work.tile([4, H], F32, tag="b1a")
        b1b = work.tile([4, H], F32, tag="b1b")
        nc.sync.dma_start(out=b1a[:], in_=bih1)
        nc.sync.dma_start(out=b1b[:], in_=bhh1)
        nc.vector.tensor_add(b1a[:], b1a[:], b1b[:])
        b1T = consts.tile([P, 4], F32)
        psb = pswt.tile([P, 4], F32, tag="pswt")
        nc.tensor.transpose(psb[:], b1a[:], ident[0:4, 0:4])
        nc.vector.tensor_copy(b1T[:], psb[:])

        # ---- x transpose prep, per quarter ----
        xT = [
            consts.tile([KA, B * TQ], F32, tag=f"xT{q}", name=f"xT{q}")
            for q in range(Q)
        ]

        def emit_xprep(q, b):
            if b == 0:
                nc.vector.memset(xT[q][:], 0.0)
                nc.vector.memset(xT[q][KA - 1 : KA, :], 1.0)
            xt = xload.tile([TQ, D], F32, tag="xt")
            nc.sync.dma_start(out=xt[:], in_=x[b, q * TQ : (q + 1) * TQ, :])
            px = psx.tile([D, TQ], F32, tag="psx")
            nc.tensor.transpose(px[:], xt[:], ident[0:TQ, 0:TQ])
            dst = xT[q][0:D, b * TQ : (b + 1) * TQ]
            if b % 2 == 0:
                nc.vector.tensor_copy(dst, px[:])
            else:
                nc.scalar.copy(out=dst, in_=px[:])

        for b in range(B):
            emit_xprep(0, b)

        # ---- main time loop; L0 at t=k, L1 at t=k-1 ----
        h0 = states.tile([P, B], F32, tag="h0")
        c0 = states.tile([P, B], F32, tag="c0")
        h1 = states.tile([P, B], F32, tag="h1")
        c1 = states.tile([P, B], F32, tag="c1")
        for t_ in (h0, c0, h1, c1):
            nc.vector.memset(t_[:], 0.0)

        # interleave next-quarter x prep into the loop, one b-tile every
        # few iterations, so PE/DVE/ACT fill chain-stall gaps with it
        prep_schedule = {}  # iter k -> list of (q, b)
        if Q > 1:
            for q in range(1, Q):
                base = (q - 1) * TQ
                for b in range(B):
                    kk = base + (b * TQ) // B
                    prep_schedule.setdefault(kk, []).append((q, b))

        for k in range(t_steps + 1):
            h0_prev, h1_prev = h0, h1
            for qb in prep_schedule.get(k, ()):
                emit_xprep(*qb)
            if k < t_steps:
                q, tl = k // TQ, k % TQ
                rhs_x = xT[q][:].rearrange("p (b t) -> p t b", t=TQ)[:, tl, :]
                ps0 = ps0p.tile([P, 4 * B], F32, tag="ps0")
                # one accumulation group per psum bank: start only on the
                # first matmul (marks the whole 2KB zero-region pending),
                # stop on the last
                for g in range(4):
                    cb = COL_OF[g] * B
                    nc.tensor.matmul(
                        ps0[:, cb : cb + B],
                        lhsT=wih0T[:, g * P : (g + 1) * P],
                        rhs=rhs_x,
                        start=(g == 0),
                        stop=False,
                    )
                for g in range(4):
                    cb = COL_OF[g] * B
                    nc.tensor.matmul(
                        ps0[:, cb : cb + B],
                        lhsT=whh0T[:, g * P : (g + 1) * P],
                        rhs=h0_prev[:],
                        start=False,
                        stop=(g == 3),
                    )
                sifo = work.tile([P, 3 * B], F32, tag="sifo")
                nc.scalar.activation(sifo[:], ps0[:, 0 : 3 * B], AF.Sigmoid)
                tg = work.tile([P, B], F32, tag="tg")
                nc.scalar.activation(tg[:], ps0[:, 3 * B : 4 * B], AF.Tanh)
                fc = work.tile([P, B], F32, tag="fc")
                nc.vector.tensor_mul(fc[:], sifo[:, B : 2 * B], c0[:])
                ig = work.tile([P, B], F32, tag="ig")
                nc.vector.tensor_mul(ig[:], sifo[:, 0:B], tg[:])
                c0 = states.tile([P, B], F32, tag="c0")
                nc.vector.tensor_add(c0[:], fc[:], ig[:])
                tc0 = work.tile([P, B], F32, tag="tc0")
                nc.scalar.activation(tc0[:], c0[:], AF.Tanh)
                h0 = states.tile([P, B], F32, tag="h0")
                nc.vector.tensor_mul(h0[:], sifo[:, 2 * B : 3 * B], tc0[:])

            if k >= 1:
                ps1 = ps1p.tile([P, 4 * B], F32, tag="ps1")
                for g in range(4):
                    cb = COL_OF[g] * B
                    nc.tensor.matmul(
                        ps1[:, cb : cb + B],
                        lhsT=wih1T[:, g * P : (g + 1) * P],
                        rhs=h0_prev[:],
                        start=(g == 0),
                        stop=False,
                    )
                for g in range(4):
                    cb = COL_OF[g] * B
                    nc.tensor.matmul(
                        ps1[:, cb : cb + B],
                        lhsT=whh1T[:, g * P : (g + 1) * P],
                        rhs=h1_prev[:],
                        start=False,
                        stop=(g == 3),
                    )
                # ACT with per-partition bias; psum col block c holds weight
                # chunk COL_OF^-1... ps1 block 0=i,1=f,2=o,3=g; bias cols by
                # weight-chunk index: i=0,f=1,g=2,o=3
                i1 = work.tile([P, B], F32, tag="i1")
                nc.scalar.activation(i1[:], ps1[:, 0:B], AF.Sigmoid, bias=b1T[:, 0:1])
                f1 = work.tile([P, B], F32, tag="f1")
                nc.scalar.activation(
                    f1[:], ps1[:, B : 2 * B], AF.Sigmoid, bias=b1T[:, 1:2]
                )
                o1 = work.tile([P, B], F32, tag="o1")
                nc.scalar.activation(
                    o1[:], ps1[:, 2 * B : 3 * B], AF.Sigmoid, bias=b1T[:, 3:4]
                )
                g1 = work.tile([P, B], F32, tag="g1")
                nc.scalar.activation(
                    g1[:], ps1[:, 3 * B : 4 * B], AF.Tanh, bias=b1T[:, 2:3]
                )
                fc1 = work.tile([P, B], F32, tag="fc1")
                nc.vector.tensor_mul(fc1[:], f1[:], c1[:])
                ig1 = work.tile([P, B], F32, tag="ig1")
                nc.vector.tensor_mul(ig1[:], i1[:], g1[:])
                c1 = states.tile([P, B], F32, tag="c1")
                nc.vector.tensor_add(c1[:], fc1[:], ig1[:])
                tc1 = work.tile([P, B], F32, tag="tc1")
                nc.scalar.activation(tc1[:], c1[:], AF.Tanh)
                h1 = states.tile([P, B], F32, tag="h1")
                nc.vector.tensor_mul(h1[:], o1[:], tc1[:])

        # ---- output: transpose h1 [128,64] -> [64,128] and store ----
        pso = ps0p.tile([B, P], F32, tag="ps0")
        nc.tensor.transpose(pso[:], h1[:], ident[:])
        ob = work.tile([B, P], F32, tag="ob")
        nc.vector.tensor_copy(ob[:], pso[:])
        nc.sync.dma_start(out=out, in_=ob[:])


_NC_CACHE = {}


def build_nc(t_steps=T_FULL):
    if t_steps in _NC_CACHE:
        return _NC_CACHE[t_steps]
    nc = bacc.Bacc(
        "TRN2",
        target_bir_lowering=False,
        debug=False,
        enable_asserts=False,
        num_devices=NCORES,
    )
    with tile.TileContext(nc) as tc:
        _emit(nc, tc, t_steps)
    nc.compile()
    _NC_CACHE[t_steps] = nc
    return nc


def make_in_maps(inputs, t_steps=T_FULL):
    x = np.asarray(inputs["x"], dtype=np.float32).reshape(B_FULL, T_FULL, D)
    x = x[:, :t_steps, :]
    shared = {
        "W_ih0": np.ascontiguousarray(inputs["W_ih0"], dtype=np.float32),
        "W_hh0": np.ascontiguousarray(inputs["W_hh0"], dtype=np.float32),
        "b_ih0": np.asarray(inputs["b_ih0"], np.float32).reshape(1, G4),
        "b_hh0": np.asarray(inputs["b_hh0"], np.float32).reshape(1, G4),
        "W_ih1": np.ascontiguousarray(inputs["W_ih1"], dtype=np.float32),
        "W_hh1": np.ascontiguousarray(inputs["W_hh1"], dtype=np.float32),
        "b_ih1": np.asarray(inputs["b_ih1"], np.float32).reshape(4, H),
        "b_hh1": np.asarray(inputs["b_hh1"], np.float32).reshape(4, H),
    }
    in_maps = []
    for c in range(NCORES):
        m = dict(shared)
        m["x"] = np.ascontiguousarray(x[c * B : (c + 1) * B])
        in_maps.append(m)
    return in_maps


def run(inputs, t_steps=T_FULL, trace=False, **kwargs):
    nc = build_nc(t_steps)
    in_maps = make_in_maps(inputs, t_steps)
    res = run_bass_kernel_spmd(
        nc, in_maps, core_ids=list(range(NCORES)), trace=trace, **kwargs
    )
    outs = [res.results[c]["out"] for c in range(NCORES)]
    return np.concatenate(outs, axis=0).astype(np.float32), res


def kernel(**inputs):
    out, _ = run(inputs)
    return out



# revision 2
# speedup vs baseline: 2.6607x; 2.6607x over previous
"""Trainium2 Bass kernel for a 2-layer LSTM extractor.

Reference computation (see problem):
  x: [512, 1, 512, 28] -> squeeze -> [B=512, T=512, D=28]
  layer0: LSTM(D=28 -> H=128), layer1: LSTM(128 -> 128)
  output: final hidden state of layer1, [512, 128]

Strategy:
  - Data parallel: batch 512 sharded 8 ways -> B=64 per NeuronCore.
  - All matmul operands in bf16 (PE runs 4x faster than fp32); PSUM
    accumulation and the c-state stay fp32.
  - All weight prep is host-side numpy: transposed lhsT chunks, L0 bias
    folded into an augmented ones-row of the x operand, x pre-transposed
    to [33, B*T] (b-major columns).
  - Per core, both layers fused in one time loop, layer1 skewed one step
    behind layer0 so its work fills engine gaps.
  - Gate-transposed layout everywhere: states h/c stored [H=128 part, B=64
    free]; gate pre-activations computed as [4H-chunk part, B free] via
    matmuls with stationary weight chunks lhsT=[K,128] and moving rhs=h.
"""

import os
import sys

import numpy as np

for _p in ("/opt/trn_rl_repo", os.path.expanduser("~/.axon_site/_ro/trn_rl_repo")):
    if os.path.isdir(_p) and _p not in sys.path:
        sys.path.insert(0, _p)

import ml_dtypes

import concourse.bacc as bacc
import concourse.tile as tile
from concourse import masks, mybir
from concourse.bass_utils import run_bass_kernel_spmd

B_FULL, T_FULL, D, H = 512, 512, 28, 128
NCORES = 8
B = B_FULL // NCORES  # 64 per core
G4 = 4 * H  # 512
P = 128
F32 = mybir.dt.float32
BF16 = mybir.dt.bfloat16
AF = mybir.ActivationFunctionType
BF16NP = ml_dtypes.bfloat16

# weight chunk g (PyTorch gate order i,f,g,o) -> psum column block.
# Sigmoid gates (i,f,o) are kept contiguous so one ACT op covers them.
COL_OF = [0, 1, 3, 2]  # i->0, f->1, g->3, o->2
KA = 33  # augmented contraction dim for the L0 x-projection (28 x + pad + bias)


def _emit(nc, tc, t_steps):
    xT_d = nc.dram_tensor("xT", [KA, B * t_steps], BF16, kind="ExternalInput").ap()
    wih0_d = nc.dram_tensor("wih0T", [KA, G4], BF16, kind="ExternalInput").ap()
    whh0_d = nc.dram_tensor("whh0T", [P, G4], BF16, kind="ExternalInput").ap()
    wih1_d = nc.dram_tensor("wih1T", [P, G4], BF16, kind="ExternalInput").ap()
    whh1_d = nc.dram_tensor("whh1T", [P, G4], BF16, kind="ExternalInput").ap()
    b1T_d = nc.dram_tensor("b1T", [P, 4], F32, kind="ExternalInput").ap()
    out = nc.dram_tensor("out", [B, H], F32, kind="ExternalOutput").ap()

    from contextlib import ExitStack

    es = ExitStack()
    with es:
        consts = es.enter_context(tc.tile_pool(name="consts", bufs=1))
        ps0p = es.enter_context(tc.tile_pool(name="ps0p", bufs=2, space="PSUM"))
        ps1p = es.enter_context(tc.tile_pool(name="ps1p", bufs=2, space="PSUM"))
        states = es.enter_context(tc.tile_pool(name="states", bufs=3))
        work = es.enter_context(tc.tile_pool(name="work", bufs=3))

        ident = consts.tile([P, P], BF16)
        masks.make_identity(nc, ident[:])

        # ---- load all pre-transposed weights + x (host-prepped, bf16) ----
        wih0T = consts.tile([KA, G4], BF16)
        whh0T = consts.tile([P, G4], BF16)
        wih1T = consts.tile([P, G4], BF16)
        whh1T = consts.tile([P, G4], BF16)
        b1T = consts.tile([P, 4], F32)
        for src, dst in (
            (wih0_d, wih0T),
            (whh0_d, whh0T),
            (wih1_d, wih1T),
            (whh1_d, whh1T),
            (b1T_d, b1T),
        ):
            nc.sync.dma_start(out=dst[:], in_=src)

        xT = consts.tile([KA, B * t_steps], BF16, name="xT")
        # split the big DMA so per-partition chunks stay < 64KB descriptors
        ncols = B * t_steps
        nchunks = max(1, ncols // 8192)
        cw = ncols // nchunks
        for i in range(nchunks):
            nc.sync.dma_start(
                out=xT[:, i * cw : (i + 1) * cw], in_=xT_d[:, i * cw : (i + 1) * cw]
            )

        # ---- main time loop; L0 at t=k, L1 at t=k-1 ----
        h0 = states.tile([P, B], BF16, tag="h0")
        c0 = states.tile([P, B], F32, tag="c0")
        h1 = states.tile([P, B], BF16, tag="h1")
        c1 = states.tile([P, B], F32, tag="c1")
        for t_ in (h0, c0, h1, c1):
            nc.vector.memset(t_[:], 0.0)
        h1f = states.tile([P, B], F32, tag="h1f")

        xT_v = xT[:].rearrange("p (b t) -> p t b", t=t_steps)

        for k in range(t_steps + 1):
            h0_prev, h1_prev = h0, h1
            if k < t_steps:
                rhs_x = xT_v[:, k, :]
                ps0 = ps0p.tile([P, 4 * B], F32, tag="ps0")
                # one accumulation group per psum bank: start only on the
                # first matmul (marks the whole 2KB zero-region pending),
                # stop on the last
                for g in range(4):
                    cb = COL_OF[g] * B
                    nc.tensor.matmul(
                        ps0[:, cb : cb + B],
                        lhsT=wih0T[:, g * P : (g + 1) * P],
                        rhs=rhs_x,
                        start=(g == 0),
                        stop=False,
                    )
                for g in range(4):
                    cb = COL_OF[g] * B
                    nc.tensor.matmul(
                        ps0[:, cb : cb + B],
                        lhsT=whh0T[:, g * P : (g + 1) * P],
                        rhs=h0_prev[:],
                        start=False,
                        stop=(g == 3),
                    )
                sifo = work.tile([P, 3 * B], F32, tag="sifo")
                nc.scalar.activation(sifo[:], ps0[:, 0 : 3 * B], AF.Sigmoid)
                tg = work.tile([P, B], F32, tag="tg")
                nc.scalar.activation(tg[:], ps0[:, 3 * B : 4 * B], AF.Tanh)
                fc = work.tile([P, B], F32, tag="fc")
                nc.vector.tensor_mul(fc[:], sifo[:, B : 2 * B], c0[:])
                ig = work.tile([P, B], F32, tag="ig")
                nc.vector.tensor_mul(ig[:], sifo[:, 0:B], tg[:])
                c0 = states.tile([P, B], F32, tag="c0")
                nc.vector.tensor_add(c0[:], fc[:], ig[:])
                tc0 = work.tile([P, B], F32, tag="tc0")
                nc.scalar.activation(tc0[:], c0[:], AF.Tanh)
                h0 = states.tile([P, B], BF16, tag="h0")
                nc.vector.tensor_mul(h0[:], sifo[:, 2 * B : 3 * B], tc0[:])

            if k >= 1:
                ps1 = ps1p.tile([P, 4 * B], F32, tag="ps1")
                for g in range(4):
                    cb = COL_OF[g] * B
                    nc.tensor.matmul(
                        ps1[:, cb : cb + B],
                        lhsT=wih1T[:, g * P : (g + 1) * P],
                        rhs=h0_prev[:],
                        start=(g == 0),
                        stop=False,
                    )
                for g in range(4):
                    cb = COL_OF[g] * B
                    nc.tensor.matmul(
                        ps1[:, cb : cb + B],
                        lhsT=whh1T[:, g * P : (g + 1) * P],
                        rhs=h1_prev[:],
                        start=False,
                        stop=(g == 3),
                    )
                # ACT with per-partition bias; ps1 col block 0=i,1=f,2=o,3=g;
                # bias cols by weight-chunk index: i=0,f=1,g=2,o=3
                i1 = work.tile([P, B], F32, tag="i1")
                nc.scalar.activation(i1[:], ps1[:, 0:B], AF.Sigmoid, bias=b1T[:, 0:1])
                f1 = work.tile([P, B], F32, tag="f1")
                nc.scalar.activation(
                    f1[:], ps1[:, B : 2 * B], AF.Sigmoid, bias=b1T[:, 1:2]
                )
                o1 = work.tile([P, B], F32, tag="o1")
                nc.scalar.activation(
                    o1[:], ps1[:, 2 * B : 3 * B], AF.Sigmoid, bias=b1T[:, 3:4]
                )
                g1 = work.tile([P, B], F32, tag="g1")
                nc.scalar.activation(
                    g1[:], ps1[:, 3 * B : 4 * B], AF.Tanh, bias=b1T[:, 2:3]
                )
                fc1 = work.tile([P, B], F32, tag="fc1")
                nc.vector.tensor_mul(fc1[:], f1[:], c1[:])
                ig1 = work.tile([P, B], F32, tag="ig1")
                nc.vector.tensor_mul(ig1[:], i1[:], g1[:])
                c1 = states.tile([P, B], F32, tag="c1")
                nc.vector.tensor_add(c1[:], fc1[:], ig1[:])
                tc1 = work.tile([P, B], F32, tag="tc1")
                nc.scalar.activation(tc1[:], c1[:], AF.Tanh)
                if k == t_steps:
                    nc.vector.tensor_mul(h1f[:], o1[:], tc1[:])
                else:
                    h1 = states.tile([P, B], BF16, tag="h1")
                    nc.vector.tensor_mul(h1[:], o1[:], tc1[:])

        # ---- output: transpose h1f [128,64] -> [64,128] and store ----
        identf = consts.tile([P, P], F32)
        masks.make_identity(nc, identf[:])
        pso = ps0p.tile([B, P], F32, tag="ps0")
        nc.tensor.transpose(pso[:], h1f[:], identf[:])
        ob = work.tile([B, P], F32, tag="ob")
        nc.vector.tensor_copy(ob[:], pso[:])
        nc.sync.dma_start(out=out, in_=ob[:])


_NC_CACHE = {}


def build_nc(t_steps=T_FULL):
    if t_steps in _NC_CACHE:
        return _NC_CACHE[t_steps]
    nc = bacc.Bacc(
        "TRN2",
        target_bir_lowering=False,
        debug=False,
        enable_asserts=False,
        num_devices=NCORES,
    )
    with tile.TileContext(nc) as tc:
        _emit(nc, tc, t_steps)
    nc.compile()
    _NC_CACHE[t_steps] = nc
    return nc


def make_in_maps(inputs, t_steps=T_FULL):
    f32 = np.float32
    x = np.asarray(inputs["x"], f32).reshape(B_FULL, T_FULL, D)[:, :t_steps, :]

    wih0T = np.zeros((KA, G4), f32)
    wih0T[:D] = np.asarray(inputs["W_ih0"], f32).T
    wih0T[KA - 1] = np.asarray(inputs["b_ih0"], f32) + np.asarray(inputs["b_hh0"], f32)

    b1 = np.asarray(inputs["b_ih1"], f32) + np.asarray(inputs["b_hh1"], f32)

    shared = {
        "wih0T": wih0T.astype(BF16NP),
        "whh0T": np.ascontiguousarray(np.asarray(inputs["W_hh0"], f32).T).astype(BF16NP),
        "wih1T": np.ascontiguousarray(np.asarray(inputs["W_ih1"], f32).T).astype(BF16NP),
        "whh1T": np.ascontiguousarray(np.asarray(inputs["W_hh1"], f32).T).astype(BF16NP),
        "b1T": np.ascontiguousarray(b1.reshape(4, H).T),
    }
    in_maps = []
    for c in range(NCORES):
        xc = x[c * B : (c + 1) * B]  # [B, t, D]
        xTc = np.zeros((KA, B * t_steps), f32)
        xTc[:D] = xc.transpose(2, 0, 1).reshape(D, B * t_steps)
        xTc[KA - 1] = 1.0
        m = dict(shared)
        m["xT"] = xTc.astype(BF16NP)
        in_maps.append(m)
    return in_maps


def run(inputs, t_steps=T_FULL, trace=False, **kwargs):
    nc = build_nc(t_steps)
    in_maps = make_in_maps(inputs, t_steps)
    res = run_bass_kernel_spmd(
        nc, in_maps, core_ids=list(range(NCORES)), trace=trace, **kwargs
    )
    outs = [res.results[c]["out"] for c in range(NCORES)]
    return np.concatenate(outs, axis=0).astype(np.float32), res


def kernel(**inputs):
    out, _ = run(inputs)
    return out


# revision 3
# speedup vs baseline: 3.7976x; 1.4273x over previous
"""Trainium2 Bass kernel for a 2-layer LSTM extractor.

Reference computation (see problem):
  x: [512, 1, 512, 28] -> squeeze -> [B=512, T=512, D=28]
  layer0: LSTM(D=28 -> H=128), layer1: LSTM(128 -> 128)
  output: final hidden state of layer1, [512, 128]

Strategy:
  - Data parallel: batch 512 sharded 8 ways -> B=64 per NeuronCore.
  - All matmul operands bf16 (PE 4x faster than fp32); PSUM accumulation
    and the c-state stay fp32.
  - Host-side numpy prep: transposed lhsT weight chunks, L0 bias folded
    into an augmented ones-row of the x operand, x pre-transposed to
    [33, B*T], L1 bias as a K=1 matmul row, g-gate weight chunks doubled.
  - The g-gate is routed through sigmoid via tanh(v) = 2*sigmoid(2v)-1
    (weights pre-doubled), so ONE sigmoid ACT op covers all 4 gates of a
    layer; the affine 2s-1 is fused into the i*g product with the custom
    DVE op affine_mul_reduce. 4 ACT ops per step total (vs 8 before).
  - Both layers fused in one time loop, layer1 skewed one step behind;
    all matmuls of an iteration are emitted before the cell math so the
    PE fills the serial-chain gaps. The L0 x-projection runs one step
    ahead into the alternate PSUM buffer.
"""

import os
import sys

import numpy as np

for _p in ("/opt/trn_rl_repo", os.path.expanduser("~/.axon_site/_ro/trn_rl_repo")):
    if os.path.isdir(_p) and _p not in sys.path:
        sys.path.insert(0, _p)

import ml_dtypes

import concourse.bacc as bacc
import concourse.tile as tile
from concourse import masks, mybir
from concourse.bass_utils import run_bass_kernel_spmd

B_FULL, T_FULL, D, H = 512, 512, 28, 128
NCORES = 8
B = B_FULL // NCORES  # 64 per core
G4 = 4 * H  # 512
P = 128
F32 = mybir.dt.float32
BF16 = mybir.dt.bfloat16
AF = mybir.ActivationFunctionType
BF16NP = ml_dtypes.bfloat16

# weight chunk g (PyTorch gate order i,f,g,o) -> psum column block.
# sigmoid gates i,f,o in blocks 0,1,2; g (pre-doubled, sigmoid'd) in block 3.
COL_OF = [0, 1, 3, 2]  # i->0, f->1, g->3, o->2
KA = 33  # augmented contraction dim for the L0 x-projection (28 x + pad + bias)


def _emit(nc, tc, t_steps):
    xT_d = nc.dram_tensor("xT", [KA, B * t_steps], BF16, kind="ExternalInput").ap()
    wih0_d = nc.dram_tensor("wih0T", [KA, G4], BF16, kind="ExternalInput").ap()
    whh0_d = nc.dram_tensor("whh0T", [P, G4], BF16, kind="ExternalInput").ap()
    wih1_d = nc.dram_tensor("wih1T", [P, G4], BF16, kind="ExternalInput").ap()
    whh1_d = nc.dram_tensor("whh1T", [P, G4], BF16, kind="ExternalInput").ap()
    b1r_d = nc.dram_tensor("b1r", [1, G4], BF16, kind="ExternalInput").ap()
    out = nc.dram_tensor("out", [B, H], F32, kind="ExternalOutput").ap()

    from contextlib import ExitStack

    es = ExitStack()
    with es:
        consts = es.enter_context(tc.tile_pool(name="consts", bufs=1))
        ps0p = es.enter_context(tc.tile_pool(name="ps0p", bufs=2, space="PSUM"))
        ps1p = es.enter_context(tc.tile_pool(name="ps1p", bufs=2, space="PSUM"))
        states = es.enter_context(tc.tile_pool(name="states", bufs=3))
        work = es.enter_context(tc.tile_pool(name="work", bufs=3))

        # ---- load all pre-transposed weights + x (host-prepped, bf16) ----
        wih0T = consts.tile([KA, G4], BF16)
        whh0T = consts.tile([P, G4], BF16)
        wih1T = consts.tile([P, G4], BF16)
        whh1T = consts.tile([P, G4], BF16)
        b1r = consts.tile([1, G4], BF16)
        for src, dst in (
            (wih0_d, wih0T),
            (whh0_d, whh0T),
            (wih1_d, wih1T),
            (whh1_d, whh1T),
            (b1r_d, b1r),
        ):
            nc.sync.dma_start(out=dst[:], in_=src)

        ones = consts.tile([1, B], BF16)
        nc.vector.memset(ones[:], 1.0)

        xT = consts.tile([KA, B * t_steps], BF16, name="xT")
        # split the big DMA so per-partition chunks stay < 64KB descriptors
        ncols = B * t_steps
        nchunks = max(1, ncols // 8192)
        cw = ncols // nchunks
        for i in range(nchunks):
            nc.sync.dma_start(
                out=xT[:, i * cw : (i + 1) * cw], in_=xT_d[:, i * cw : (i + 1) * cw]
            )

        # ---- states ----
        h0 = states.tile([P, B], BF16, tag="h0")
        c0 = states.tile([P, B], F32, tag="c0")
        h1 = states.tile([P, B], BF16, tag="h1")
        c1 = states.tile([P, B], F32, tag="c1")
        for t_ in (h0, c0, h1, c1):
            nc.vector.memset(t_[:], 0.0)
        h1f = states.tile([P, B], F32, tag="h1f")
        acc0 = states.tile([P, 1], F32, tag="acc0")  # dummy accum for custom dve
        acc1 = states.tile([P, 1], F32, tag="acc1")

        xT_v = xT[:].rearrange("p (b t) -> p t b", t=t_steps)

        def emit_xproj(ps, k):
            # starts the accumulation group for step k's L0 psum bank
            rhs_x = xT_v[:, k, :]
            for g in range(4):
                cb = COL_OF[g] * B
                nc.tensor.matmul(
                    ps[:, cb : cb + B],
                    lhsT=wih0T[:, g * P : (g + 1) * P],
                    rhs=rhs_x,
                    start=(g == 0),
                    stop=False,
                )

        # prologue: x-projection for step 0
        ps0 = ps0p.tile([P, 4 * B], F32, tag="ps0")
        emit_xproj(ps0, 0)

        for k in range(t_steps + 1):
            h0_prev, h1_prev = h0, h1
            # ---- all matmuls first (PE fills chain gaps) ----
            if k < t_steps:
                for g in range(4):  # L0 recurrent; closes step-k group
                    cb = COL_OF[g] * B
                    nc.tensor.matmul(
                        ps0[:, cb : cb + B],
                        lhsT=whh0T[:, g * P : (g + 1) * P],
                        rhs=h0_prev[:],
                        start=False,
                        stop=(g == 3),
                    )
                if k + 1 < t_steps:
                    ps0_next = ps0p.tile([P, 4 * B], F32, tag="ps0")
                    emit_xproj(ps0_next, k + 1)
            if k >= 1:
                ps1 = ps1p.tile([P, 4 * B], F32, tag="ps1")
                for g in range(4):  # bias row via K=1 matmul (starts group)
                    cb = COL_OF[g] * B
                    nc.tensor.matmul(
                        ps1[:, cb : cb + B],
                        lhsT=b1r[0:1, g * P : (g + 1) * P],
                        rhs=ones[:],
                        start=(g == 0),
                        stop=False,
                    )
                for g in range(4):
                    cb = COL_OF[g] * B
                    nc.tensor.matmul(
                        ps1[:, cb : cb + B],
                        lhsT=wih1T[:, g * P : (g + 1) * P],
                        rhs=h0_prev[:],
                        start=False,
                        stop=False,
                    )
                for g in range(4):
                    cb = COL_OF[g] * B
                    nc.tensor.matmul(
                        ps1[:, cb : cb + B],
                        lhsT=whh1T[:, g * P : (g + 1) * P],
                        rhs=h1_prev[:],
                        start=False,
                        stop=(g == 3),
                    )

            # ---- gate sigmoids (one ACT op per layer) ----
            if k < t_steps:
                sig0 = work.tile([P, 4 * B], F32, tag="sig0")
                nc.scalar.activation(sig0[:], ps0[:], AF.Sigmoid)
            if k >= 1:
                sig1 = work.tile([P, 4 * B], F32, tag="sig1")
                nc.scalar.activation(sig1[:], ps1[:], AF.Sigmoid)

            # ---- L0 cell update ----
            if k < t_steps:
                fc = work.tile([P, B], F32, tag="fc")
                nc.vector.tensor_mul(fc[:], sig0[:, B : 2 * B], c0[:])
                ig = work.tile([P, B], F32, tag="ig")
                nc.vector.affine_mul_reduce(
                    ig[:], acc0[:], sig0[:, 3 * B : 4 * B], sig0[:, 0:B], 2.0, -1.0
                )
                c0 = states.tile([P, B], F32, tag="c0")
                nc.vector.tensor_add(c0[:], fc[:], ig[:])
                tc0 = work.tile([P, B], F32, tag="tc0")
                nc.scalar.activation(tc0[:], c0[:], AF.Tanh)
                h0 = states.tile([P, B], BF16, tag="h0")
                nc.vector.tensor_mul(h0[:], sig0[:, 2 * B : 3 * B], tc0[:])
                ps0 = ps0_next

            # ---- L1 cell update (step k-1) ----
            if k >= 1:
                fc1 = work.tile([P, B], F32, tag="fc1")
                nc.vector.tensor_mul(fc1[:], sig1[:, B : 2 * B], c1[:])
                ig1 = work.tile([P, B], F32, tag="ig1")
                nc.vector.affine_mul_reduce(
                    ig1[:], acc1[:], sig1[:, 3 * B : 4 * B], sig1[:, 0:B], 2.0, -1.0
                )
                c1 = states.tile([P, B], F32, tag="c1")
                nc.vector.tensor_add(c1[:], fc1[:], ig1[:])
                tc1 = work.tile([P, B], F32, tag="tc1")
                nc.scalar.activation(tc1[:], c1[:], AF.Tanh)
                if k == t_steps:
                    nc.vector.tensor_mul(h1f[:], sig1[:, 2 * B : 3 * B], tc1[:])
                else:
                    h1 = states.tile([P, B], BF16, tag="h1")
                    nc.vector.tensor_mul(h1[:], sig1[:, 2 * B : 3 * B], tc1[:])

        # ---- output: transpose h1f [128,64] -> [64,128] and store ----
        identf = consts.tile([P, P], F32)
        masks.make_identity(nc, identf[:])
        pso = ps0p.tile([B, P], F32, tag="pso")
        nc.tensor.transpose(pso[:], h1f[:], identf[:])
        ob = work.tile([B, P], F32, tag="ob")
        nc.vector.tensor_copy(ob[:], pso[:])
        nc.sync.dma_start(out=out, in_=ob[:])


_NC_CACHE = {}


def build_nc(t_steps=T_FULL):
    if t_steps in _NC_CACHE:
        return _NC_CACHE[t_steps]
    nc = bacc.Bacc(
        "TRN2",
        target_bir_lowering=False,
        debug=False,
        enable_asserts=False,
        num_devices=NCORES,
    )
    with tile.TileContext(nc) as tc:
        _emit(nc, tc, t_steps)
    nc.compile()
    _NC_CACHE[t_steps] = nc
    return nc


def make_in_maps(inputs, t_steps=T_FULL):
    f32 = np.float32
    x = np.asarray(inputs["x"], f32).reshape(B_FULL, T_FULL, D)[:, :t_steps, :]

    # g-gate chunk (PyTorch order i,f,g,o -> chunk 2) weights and biases are
    # doubled so sigmoid(2v) recovers tanh(v) = 2*sigmoid(2v)-1.
    gsl = slice(2 * H, 3 * H)

    wih0T = np.zeros((KA, G4), f32)
    wih0T[:D] = np.asarray(inputs["W_ih0"], f32).T
    wih0T[KA - 1] = np.asarray(inputs["b_ih0"], f32) + np.asarray(inputs["b_hh0"], f32)
    wih0T[:, gsl] *= 2.0

    whh0T = np.ascontiguousarray(np.asarray(inputs["W_hh0"], f32).T)
    whh0T[:, gsl] *= 2.0
    wih1T = np.ascontiguousarray(np.asarray(inputs["W_ih1"], f32).T)
    wih1T[:, gsl] *= 2.0
    whh1T = np.ascontiguousarray(np.asarray(inputs["W_hh1"], f32).T)
    whh1T[:, gsl] *= 2.0

    b1 = np.asarray(inputs["b_ih1"], f32) + np.asarray(inputs["b_hh1"], f32)
    b1[gsl] *= 2.0

    shared = {
        "wih0T": wih0T.astype(BF16NP),
        "whh0T": whh0T.astype(BF16NP),
        "wih1T": wih1T.astype(BF16NP),
        "whh1T": whh1T.astype(BF16NP),
        "b1r": b1.reshape(1, G4).astype(BF16NP),
    }
    in_maps = []
    for c in range(NCORES):
        xc = x[c * B : (c + 1) * B]  # [B, t, D]
        xTc = np.zeros((KA, B * t_steps), f32)
        xTc[:D] = xc.transpose(2, 0, 1).reshape(D, B * t_steps)
        xTc[KA - 1] = 1.0
        m = dict(shared)
        m["xT"] = xTc.astype(BF16NP)
        in_maps.append(m)
    return in_maps


def run(inputs, t_steps=T_FULL, trace=False, **kwargs):
    nc = build_nc(t_steps)
    in_maps = make_in_maps(inputs, t_steps)
    res = run_bass_kernel_spmd(
        nc, in_maps, core_ids=list(range(NCORES)), trace=trace, **kwargs
    )
    outs = [res.results[c]["out"] for c in range(NCORES)]
    return np.concatenate(outs, axis=0).astype(np.float32), res


def kernel(**inputs):
    out, _ = run(inputs)
    return out
